# revision 19
# baseline (speedup 1.0000x reference)
"""Trainium2 Bass kernel for a 3-layer GraphSAGE GNN (CellTypeGNN).

Strategy (graph/data parallel over 8 NeuronCores):
- Nodes are sharded by range: core c owns nodes [c*6250, (c+1)*6250).
- Edges are assigned to the core owning their destination node, grouped into
  128-node destination windows, and packed into 128-edge subtiles.
- Messages x[src] are fetched with batched SWDGE dma_gather (fp16, 256B rows)
  from a full replica of the current features in DRAM, built on device by an
  AllGather of the per-core fp16 node shard (layer 1 included, so the host
  only ever uploads the local shard). int16 gather indices only reach 32767,
  so the table is split into lo (nodes < 25000) and hi halves; edges are
  segregated by source half within each window.
- Segment-mean aggregation: for each 128-edge subtile an fp16 one-hot matrix
  scaled by 1/deg(dst) is built on the vector engine with a single
  tensor_scalar(is_equal, mult); PE matmuls msg^T @ onehot accumulate the mean
  into PSUM per destination window, feature-major.
- SAGE linear: fp16 weight matmuls (Wl on aggregated mean + Wr on x) into the
  same PSUM bank; LayerNorm stats computed node-major (PE transpose),
  affine+GELU fused into one scalar-engine activation, residual on DVE.
- After layers 1 and 2 the updated fp16 node features are AllGathered across
  the 8 cores so the next layer can gather arbitrary source nodes.
- Classifier LayerNorm affine is folded into the final linear on the host.
- Per-call host->device staging cost in this environment is dominated by the
  NUMBER of runtime input buffers (~1.7 ms each), so ALL per-core inputs are
  packed into one int16 blob (x shard fp16, compact 16-row wrapped gather
  indices, rel/inv edge tables fp16, packed weights) and unpacked on device:
  indices are replicated 16->128 rows with DRAM->DRAM DMAs, rel/inv converted
  to fp32 on DVE, iota/identity matrices generated with gpsimd iota.
"""

import numpy as np
from contextlib import ExitStack

import concourse.bass as bass
import concourse.tile as tile
from concourse import bacc, mybir
from concourse.bass_utils import run_bass_kernel_spmd

P = 128
N, E, D = 50000, 800000, 128
DOUT, NCLS = 64, 40
NCORES = 8
NPC = N // NCORES            # 6250 nodes per core
W = (NPC + P - 1) // P       # 49 windows per core
NPAD = W * P                 # 6272 padded per-core node count
HALF = 25000                 # lo/hi table split (int16 gather indices)
G = 32                       # gather chunk size in 128-edge subtiles
LN_EPS = 1e-5
W16COLS = 4 * 128 + 2 * 64 + NCLS   # packed fp16 weights: wl0 wr0 wl1 wr1 wl2 wr2 wc
W32COLS = 9                          # bl0 bl1 bl2 g0 b0 g1 b1 bc eps

f32 = mybir.dt.float32
f16 = mybir.dt.float16
i16 = mybir.dt.int16
i8 = mybir.dt.int8

_cache = {}


def _layout(TL, TH):
    """Blob segment layout in int16 elements. Every segment size is a
    multiple of 64 so offsets stay 128B aligned."""
    T2 = TL + TH
    segs = {}
    off = 0

    def add(name, rows, cols, dt):
        nonlocal off
        if dt == f32:
            n = rows * cols * 2
        elif dt == i8:
            n = (rows * cols + 1) // 2
        else:
            n = rows * cols
        segs[name] = (off, rows, cols, dt)
        off += (n + 63) & ~63

    add("xsh", NPC, D, f16)
    add("idxA", 16, TL * 8, i16)
    add("idxB", 16, TH * 8, i16)
    add("rel8", P, T2, i8)
    add("inv16", P, T2, f16)
    add("wts16", P, W16COLS, f16)
    add("wts32", P, W32COLS, f32)
    segs["total"] = off
    return segs


def _schedule(edge_index):
    """Host-side edge preprocessing. Returns per-core gather/one-hot arrays
    plus the SPMD-uniform window subtile schedule."""
    src = edge_index[0].astype(np.int64)
    dst = edge_index[1].astype(np.int64)
    deg = np.bincount(dst, minlength=N)
    invdeg_all = (1.0 / np.maximum(deg, 1)).astype(np.float32)

    core = dst // NPC
    loc = dst - core * NPC
    win = loc >> 7
    rel = (loc & 127).astype(np.float32)
    half = (src >= HALF).astype(np.int64)

    # group id: (core, window, half); edges sorted by group
    gid = (core * W + win) * 2 + half
    order = np.argsort(gid, kind="stable")
    gid_s = gid[order]
    counts = np.bincount(gid_s, minlength=NCORES * W * 2).reshape(NCORES, W, 2)

    # SPMD-uniform subtile counts per (window, half)
    ntiles = (counts.max(axis=0) + P - 1) // P  # [W, 2]
    empty = ntiles.sum(axis=1) == 0
    ntiles[empty, 0] = 1
    TL = int(ntiles[:, 0].sum())
    TH = int(ntiles[:, 1].sum())
    startA = np.concatenate([[0], np.cumsum(ntiles[:, 0])[:-1]]).astype(np.int64)
    startB = np.concatenate([[0], np.cumsum(ntiles[:, 1])[:-1]]).astype(np.int64)

    # per-core stream arrays
    idxA = np.zeros((NCORES, P, TL), np.int16)
    idxB = np.zeros((NCORES, P, TH), np.int16)
    relA = np.full((NCORES, P, TL), -1.0, np.float32)
    relB = np.full((NCORES, P, TH), -1.0, np.float32)
    invA = np.zeros((NCORES, P, TL), np.float32)
    invB = np.zeros((NCORES, P, TH), np.float32)

    # vectorized placement: rank of each edge within its (core, win, half) group
    grp_start_per_edge = np.concatenate([[0], np.cumsum(np.bincount(
        gid_s, minlength=NCORES * W * 2))])[gid_s]
    rank = np.arange(len(gid_s)) - grp_start_per_edge
    e_core = core[order]
    e_win = win[order]
    e_half = half[order]
    e_src = src[order]
    e_rel = rel[order]
    e_inv = invdeg_all[dst[order]]
    pos = np.where(e_half == 0, startA[e_win], startB[e_win]) + (rank >> 7)
    prt = rank & 127

    mA = e_half == 0
    idxA[e_core[mA], prt[mA], pos[mA]] = e_src[mA].astype(np.int16)
    relA[e_core[mA], prt[mA], pos[mA]] = e_rel[mA]
    invA[e_core[mA], prt[mA], pos[mA]] = e_inv[mA]
    mB = ~mA
    idxB[e_core[mB], prt[mB], pos[mB]] = (e_src[mB] - HALF).astype(np.int16)
    relB[e_core[mB], prt[mB], pos[mB]] = e_rel[mB]
    invB[e_core[mB], prt[mB], pos[mB]] = e_inv[mB]

    def wrap16(idx_pt):  # [P, T] slot-major -> compact wrapped [16, T*8]
        Tn = idx_pt.shape[1]
        out = np.zeros((16, Tn * 8), np.int16)
        for c0 in range(0, Tn, G):
            c1 = min(c0 + G, Tn)
            flat = idx_pt[:, c0:c1].T.reshape(-1)  # i = t_local*128 + p
            w16 = flat.reshape(-1, 16).T  # [16, n/16]: i -> [i%16, i//16]
            out[:, c0 * 8 : c0 * 8 + w16.shape[1]] = w16
        return out

    idxAw = np.stack([wrap16(idxA[c]) for c in range(NCORES)])
    idxBw = np.stack([wrap16(idxB[c]) for c in range(NCORES)])
    return dict(
        ntiles=ntiles, TL=TL, TH=TH, startA=startA, startB=startB,
        idxA=idxAw, idxB=idxBw, relA=relA, relB=relB, invA=invA, invB=invB,
    )


def _build(sched, krep=1):
    """Build and compile the SPMD Bass program."""
    ntiles, TL, TH = sched["ntiles"], sched["TL"], sched["TH"]
    startA, startB = sched["startA"], sched["startB"]
    T2 = TL + TH
    lay = _layout(TL, TH)

    nc = bacc.Bacc("TRN2", target_bir_lowering=False, debug=False,
                   num_devices=NCORES)

    blob_d = nc.dram_tensor("blob", [lay["total"]], i16, kind="ExternalInput")
    out_d = nc.dram_tensor("out", [NCLS, NPAD], f16, kind="ExternalOutput")

    idxA_rep = nc.dram_tensor("idxA_rep", [P, TL * 8], i16)
    idxB_rep = nc.dram_tensor("idxB_rep", [P, TH * 8], i16)
    xg_sh = nc.dram_tensor("xg_sh", [NPC, D], f16)
    xg_fullx = nc.dram_tensor("xg_fullx", [N, D], f16, addr_space="Shared")
    xg_own = [nc.dram_tensor(f"xg{l}_own", [NPC, D], f16) for l in range(2)]
    xg_full = [
        nc.dram_tensor(f"xg{l}_full", [N, D], f16, addr_space="Shared")
        for l in range(2)
    ]

    def seg_ap(name):
        off, rows, cols, dt = lay[name]
        if dt == f32:
            n = rows * cols * 2
        elif dt == i8:
            n = (rows * cols + 1) // 2
        else:
            n = rows * cols
        a = blob_d.ap()[off : off + n]
        if dt != i16:
            a = a.bitcast(dt)
        return a.rearrange("(r c) -> r c", c=cols)

    with tile.TileContext(nc) as tc, ExitStack() as ctx:
        cpool = ctx.enter_context(tc.tile_pool(name="const", bufs=1))
        xpool = ctx.enter_context(tc.tile_pool(name="x", bufs=1))
        idxpool = ctx.enter_context(tc.tile_pool(name="idx", bufs=4))
        msgApool = ctx.enter_context(tc.tile_pool(name="msgA", bufs=3))
        msgBpool = ctx.enter_context(tc.tile_pool(name="msgB", bufs=3))
        ohpool = ctx.enter_context(tc.tile_pool(name="oh", bufs=10))
        wkpool = ctx.enter_context(tc.tile_pool(name="wk", bufs=4))
        stpool = ctx.enter_context(tc.tile_pool(name="st", bufs=8))
        psA = ctx.enter_context(tc.tile_pool(name="psA", bufs=3, space="PSUM"))
        psC = ctx.enter_context(tc.tile_pool(name="psC", bufs=2, space="PSUM"))
        psT = ctx.enter_context(tc.tile_pool(name="psT", bufs=3, space="PSUM"))

        # ---- preamble: unpack the blob --------------------------------
        # layer-1 gather table: AllGather the fp16 shard
        nc.sync.dma_start(out=xg_sh.ap(), in_=seg_ap("xsh"))
        nc.gpsimd.collective_compute(
            "AllGather",
            mybir.AluOpType.bypass,
            replica_groups=[list(range(NCORES))],
            ins=[xg_sh.ap()],
            outs=[xg_fullx.ap()],
        )

        # replicate compact 16-row wrapped indices to the 128-row form the
        # SWDGE gather expects (DRAM->DRAM)
        segA, segB = seg_ap("idxA"), seg_ap("idxB")
        for k in range(8):
            nc.sync.dma_start(out=idxA_rep.ap()[16 * k : 16 * (k + 1), :], in_=segA)
            nc.sync.dma_start(out=idxB_rep.ap()[16 * k : 16 * (k + 1), :], in_=segB)

        # packed weights
        w16 = cpool.tile([P, W16COLS], f16, name="w16")
        nc.sync.dma_start(out=w16[:], in_=seg_ap("wts16"))
        w32 = cpool.tile([P, W32COLS], f32, name="w32")
        nc.sync.dma_start(out=w32[:], in_=seg_ap("wts32"))
        wl_v = [w16[:, 0:128], w16[:, 256:384], w16[:, 512:576]]
        wr_v = [w16[:, 128:256], w16[:, 384:512], w16[:, 576:640]]
        wc_v = w16[:DOUT, 640 : 640 + NCLS]
        bl_v = [w32[:, 0:1], w32[:, 1:2], w32[:DOUT, 2:3]]
        g_v = [w32[:, 3:4], w32[:, 5:6]]
        b_v = [w32[:, 4:5], w32[:, 6:7]]
        bc_v = w32[:NCLS, 7:8]
        eps_v = w32[:, 8:9]

        # rel/inv tables: int8/fp16 upload -> fp32 working copy
        r8 = cpool.tile([P, T2], i8, name="r8")
        nc.sync.dma_start(out=r8[:], in_=seg_ap("rel8"))
        iv16 = cpool.tile([P, T2], f16, name="iv16")
        nc.sync.dma_start(out=iv16[:], in_=seg_ap("inv16"))
        conv = cpool.tile([P, 2 * T2], f32, name="conv")
        nc.any.tensor_copy(conv[:, :T2], r8[:])
        nc.any.tensor_copy(conv[:, T2:], iv16[:])
        # conv columns: [relA TL][relB TH][invA TL][invB TH]

        # iota / identity constants, generated on device
        iota16 = cpool.tile([P, P], f16, name="iota16")
        nc.gpsimd.iota(iota16[:], [[1, P]], channel_multiplier=0,
                       allow_small_or_imprecise_dtypes=True)
        iota32 = cpool.tile([P, P], f32, name="iota32")
        nc.gpsimd.iota(iota32[:], [[1, P]], channel_multiplier=0,
                       allow_small_or_imprecise_dtypes=True)
        pidx = cpool.tile([P, 1], f32, name="pidx")
        nc.gpsimd.iota(pidx[:], [[1, 1]], channel_multiplier=1,
                       allow_small_or_imprecise_dtypes=True)
        ident32 = cpool.tile([P, P], f32, name="ident32")
        nc.vector.tensor_scalar(out=ident32[:], in0=iota32[:],
                                scalar1=pidx[:, :1], scalar2=None,
                                op0=mybir.AluOpType.is_equal)

        # feature-major working copies of the local shard
        xfm32 = [xpool.tile([P, NPAD], f32, tag=f"xfm32_{i}", name=f"xfm32_{i}")
                 for i in range(2)]
        xfm16 = [xpool.tile([P, NPAD], f16, tag=f"xfm16_{i}", name=f"xfm16_{i}")
                 for i in range(2)]
        xsha = seg_ap("xsh")
        for w in range(W):
            rows = min(P, NPC - w * P)
            nm = wkpool.tile([P, P], f16, tag="xnm_in")
            nc.sync.dma_start(out=nm[:rows, :], in_=xsha[w * P : w * P + rows, :])
            nm32 = wkpool.tile([P, P], f32, tag="xnm32")
            nc.any.tensor_copy(nm32[:rows, :], nm[:rows, :])
            tp = psT.tile([P, P], f32, space="PSUM", tag="tp")
            nc.tensor.transpose(tp[:, :rows], nm32[:rows, :], ident32[:rows, :rows])
            nc.any.tensor_copy(xfm32[0][:, w * P : w * P + rows], tp[:, :rows])
            nc.any.tensor_copy(xfm16[0][:, w * P : w * P + rows], tp[:, :rows])
        if NPC < NPAD:
            nc.vector.memset(xfm32[0][:, NPC:NPAD], 0)
            nc.vector.memset(xfm16[0][:, NPC:NPAD], 0)

        x3fm = xpool.tile([DOUT, NPAD], f32, tag="x3fm")
        normfm = xpool.tile([DOUT, NPAD], f16, tag="normfm")

        nchunkA = (TL + G - 1) // G
        nchunkB = (TH + G - 1) // G

        for l in list(range(3)) * krep:
            dout = D if l < 2 else DOUT
            cur32, cur16 = xfm32[l % 2], xfm16[l % 2]
            nxt32, nxt16 = xfm32[(l + 1) % 2], xfm16[(l + 1) % 2]
            gsrc = xg_fullx if l == 0 else xg_full[l - 1]
            src_lo = gsrc.ap()[:HALF, :]
            src_hi = gsrc.ap()[HALF:, :]

            # emit gather chunks lazily; Tile pool backpressure pipelines them
            msgs = {"A": {}, "B": {}}
            issued = {"A": -1, "B": -1}

            def emit_chunk(stream, ci, l=l, src_lo=src_lo, src_hi=src_hi,
                           msgs=msgs):
                Tn = TL if stream == "A" else TH
                idxd = idxA_rep if stream == "A" else idxB_rep
                mpool = msgApool if stream == "A" else msgBpool
                src = src_lo if stream == "A" else src_hi
                c0 = ci * G
                cn = min(G, Tn - c0)
                nidx = cn * P
                it = idxpool.tile([P, G * 8], i16, tag="idx")
                nc.sync.dma_start(
                    out=it[:, : cn * 8], in_=idxd.ap()[:, c0 * 8 : c0 * 8 + cn * 8]
                )
                mt = mpool.tile([P, G * P], f16, tag=f"msg{stream}")
                nc.gpsimd.dma_gather(
                    mt[:, : cn * P].rearrange("p (t d) -> p t d", d=P),
                    src,
                    it[:, : cn * 8],
                    nidx,
                    nidx,
                    P,
                    single_packet=False,
                )
                msgs[stream][ci] = mt

            for w in range(W):
                nA, nB = int(ntiles[w, 0]), int(ntiles[w, 1])
                subs = [("A", int(startA[w]) + i) for i in range(nA)] + [
                    ("B", int(startB[w]) + i) for i in range(nB)
                ]
                for stream, pos in subs:
                    while issued[stream] < pos // G:
                        issued[stream] += 1
                        emit_chunk(stream, issued[stream])

                ps = psA.tile([P, P], f32, space="PSUM", tag="agg")
                for si, (stream, pos) in enumerate(subs):
                    rcol = pos if stream == "A" else TL + pos
                    icol = T2 + rcol
                    mt = msgs[stream][pos // G]
                    t = pos % G
                    oh = ohpool.tile([P, P], f16, tag="oh")
                    nc.vector.tensor_scalar(
                        out=oh[:],
                        in0=iota16[:],
                        scalar1=conv[:, rcol : rcol + 1],
                        scalar2=conv[:, icol : icol + 1],
                        op0=mybir.AluOpType.is_equal,
                        op1=mybir.AluOpType.mult,
                    )
                    nc.tensor.matmul(
                        out=ps[:],
                        lhsT=mt[:, t * P : (t + 1) * P],
                        rhs=oh[:],
                        start=(si == 0),
                        stop=(si == len(subs) - 1),
                    )
                agg16 = wkpool.tile([P, P], f16, tag="agg16")
                nc.any.tensor_copy(agg16[:], ps[:])

                hps = psC.tile([dout, P], f32, space="PSUM", tag="h")
                nc.tensor.matmul(out=hps[:], lhsT=wl_v[l], rhs=agg16[:],
                                 start=True, stop=False)
                nc.tensor.matmul(out=hps[:], lhsT=wr_v[l],
                                 rhs=cur16[:, w * P : (w + 1) * P],
                                 start=False, stop=True)

                cols = slice(w * P, (w + 1) * P)
                if l < 2:
                    hfm = wkpool.tile([P, P], f32, tag="hfm")
                    nc.scalar.activation(hfm[:], hps[:],
                                         mybir.ActivationFunctionType.Identity,
                                         bias=bl_v[l])
                    tp1 = psT.tile([P, P], f32, space="PSUM", tag="tp")
                    nc.tensor.transpose(tp1[:], hfm[:], ident32[:])
                    s_ = stpool.tile([P, 1], f32, tag="sum")
                    nc.vector.reduce_sum(s_[:], tp1[:], axis=mybir.AxisListType.X)
                    nmu = stpool.tile([P, 1], f32, tag="nmu")
                    nc.scalar.mul(nmu[:], s_[:], -1.0 / D)
                    xc = wkpool.tile([P, P], f32, tag="xc")
                    nc.scalar.activation(xc[:], tp1[:],
                                         mybir.ActivationFunctionType.Identity,
                                         bias=nmu[:, :1])
                    sq = wkpool.tile([P, P], f32, tag="sq")
                    ss = stpool.tile([P, 1], f32, tag="ss")
                    nc.scalar.activation(sq[:], xc[:],
                                         mybir.ActivationFunctionType.Square,
                                         accum_out=ss[:, :1])
                    sd = stpool.tile([P, 1], f32, tag="sd")
                    nc.scalar.activation(sd[:], ss[:],
                                         mybir.ActivationFunctionType.Sqrt,
                                         scale=1.0 / D, bias=eps_v)
                    rs = stpool.tile([P, 1], f32, tag="rs")
                    nc.vector.reciprocal(rs[:], sd[:])
                    nrm = wkpool.tile([P, P], f32, tag="nrm")
                    nc.vector.tensor_scalar_mul(nrm[:], xc[:], rs[:, :1])
                    tp2 = psT.tile([P, P], f32, space="PSUM", tag="tp")
                    nc.tensor.transpose(tp2[:], nrm[:], ident32[:])
                    gel = wkpool.tile([P, P], f32, tag="gel")
                    nc.scalar.activation(gel[:], tp2[:],
                                         mybir.ActivationFunctionType.Gelu,
                                         bias=b_v[l], scale=g_v[l])
                    nc.vector.tensor_add(nxt32[:, cols], gel[:], cur32[:, cols])
                    nc.any.tensor_copy(nxt16[:, cols], nxt32[:, cols])
                    tp3 = psT.tile([P, P], f32, space="PSUM", tag="tp")
                    nc.tensor.transpose(tp3[:], nxt32[:, cols], ident32[:])
                    xnm = wkpool.tile([P, P], f16, tag="xnm")
                    nc.any.tensor_copy(xnm[:], tp3[:])
                    rows = min(P, NPC - w * P)
                    nc.sync.dma_start(
                        out=xg_own[l].ap()[w * P : w * P + rows, :],
                        in_=xnm[:rows, :],
                    )
                else:
                    nc.scalar.activation(x3fm[:, cols], hps[:],
                                         mybir.ActivationFunctionType.Gelu,
                                         bias=bl_v[l])

            if l < 2:
                nc.gpsimd.collective_compute(
                    "AllGather",
                    mybir.AluOpType.bypass,
                    replica_groups=[list(range(NCORES))],
                    ins=[xg_own[l].ap()],
                    outs=[xg_full[l].ap()],
                )

        # classifier: LN (affine folded into wc) then linear
        for w in range(W):
            cols = slice(w * P, (w + 1) * P)
            tp1 = psT.tile([P, DOUT], f32, space="PSUM", tag="tp")
            nc.tensor.transpose(tp1[:], x3fm[:, cols], ident32[:DOUT, :DOUT])
            s_ = stpool.tile([P, 1], f32, tag="sum")
            nc.vector.reduce_sum(s_[:], tp1[:], axis=mybir.AxisListType.X)
            nmu = stpool.tile([P, 1], f32, tag="nmu")
            nc.scalar.mul(nmu[:], s_[:], -1.0 / DOUT)
            xc = wkpool.tile([P, DOUT], f32, tag="xc")
            nc.scalar.activation(xc[:], tp1[:],
                                 mybir.ActivationFunctionType.Identity,
                                 bias=nmu[:, :1])
            sq = wkpool.tile([P, DOUT], f32, tag="sq")
            ss = stpool.tile([P, 1], f32, tag="ss")
            nc.scalar.activation(sq[:], xc[:],
                                 mybir.ActivationFunctionType.Square,
                                 accum_out=ss[:, :1])
            sd = stpool.tile([P, 1], f32, tag="sd")
            nc.scalar.activation(sd[:], ss[:],
                                 mybir.ActivationFunctionType.Sqrt,
                                 scale=1.0 / DOUT, bias=eps_v)
            rs = stpool.tile([P, 1], f32, tag="rs")
            nc.vector.reciprocal(rs[:], sd[:])
            nrm = wkpool.tile([P, DOUT], f32, tag="nrm")
            nc.vector.tensor_scalar_mul(nrm[:], xc[:], rs[:, :1])
            tp2 = psT.tile([DOUT, P], f32, space="PSUM", tag="tp")
            nc.tensor.transpose(tp2[:], nrm[:], ident32[:])
            nc.any.tensor_copy(normfm[:, cols], tp2[:])

        NCHUNK = 512
        for c0 in range(0, NPAD, NCHUNK):
            cn = min(NCHUNK, NPAD - c0)
            ops = psC.tile([NCLS, NCHUNK], f32, space="PSUM", tag="h")
            nc.tensor.matmul(out=ops[:, :cn], lhsT=wc_v,
                             rhs=normfm[:, c0 : c0 + cn], start=True, stop=True)
            osb = wkpool.tile([NCLS, NCHUNK], f16, tag="osb")
            nc.scalar.activation(osb[:, :cn], ops[:, :cn],
                                 mybir.ActivationFunctionType.Identity,
                                 bias=bc_v)
            nc.sync.dma_start(out=out_d.ap()[:, c0 : c0 + cn], in_=osb[:, :cn])

    nc.compile()
    return nc


def _prep_inputs(x, sched, weights):
    """Pack per-core input blobs."""
    TL, TH = sched["TL"], sched["TH"]
    T2 = TL + TH
    lay = _layout(TL, TH)
    (Wl1, bl1, Wr1, g1, b1, Wl2, bl2, Wr2, g2, b2,
     Wl3, bl3, Wr3, gc, bc, Wc, bcls) = weights
    wcp = (gc[:, None].astype(np.float32) * Wc.astype(np.float32))
    bcp = bc.astype(np.float32) @ Wc.astype(np.float32) + bcls.astype(np.float32)

    wts16 = np.zeros((P, W16COLS), np.float16)
    wts16[:, 0:128] = Wl1.astype(np.float16)
    wts16[:, 128:256] = Wr1.astype(np.float16)
    wts16[:, 256:384] = Wl2.astype(np.float16)
    wts16[:, 384:512] = Wr2.astype(np.float16)
    wts16[:, 512:576] = Wl3.astype(np.float16)
    wts16[:, 576:640] = Wr3.astype(np.float16)
    wts16[:DOUT, 640 : 640 + NCLS] = wcp.astype(np.float16)

    wts32 = np.zeros((P, W32COLS), np.float32)
    wts32[:, 0] = bl1.astype(np.float32)
    wts32[:, 1] = bl2.astype(np.float32)
    wts32[:DOUT, 2] = bl3.astype(np.float32)
    wts32[:, 3] = g1.astype(np.float32)
    wts32[:, 4] = b1.astype(np.float32)
    wts32[:, 5] = g2.astype(np.float32)
    wts32[:, 6] = b2.astype(np.float32)
    wts32[:NCLS, 7] = bcp
    wts32[:, 8] = LN_EPS

    def place(blob, name, arr):
        off = lay[name][0]
        a = arr.reshape(-1).view(np.int16)
        blob[off : off + a.size] = a

    in_maps = []
    for c in range(NCORES):
        blob = np.zeros(lay["total"], np.int16)
        place(blob, "xsh", np.ascontiguousarray(
            x[c * NPC : (c + 1) * NPC].astype(np.float16)))
        place(blob, "idxA", sched["idxA"][c])
        place(blob, "idxB", sched["idxB"][c])
        rel8 = np.empty((P, T2), np.int8)
        rel8[:, 0:TL] = sched["relA"][c]
        rel8[:, TL:T2] = sched["relB"][c]
        place(blob, "rel8", rel8)
        inv16 = np.empty((P, T2), np.float16)
        inv16[:, 0:TL] = sched["invA"][c]
        inv16[:, TL:T2] = sched["invB"][c]
        place(blob, "inv16", inv16)
        place(blob, "wts16", wts16)
        place(blob, "wts32", wts32)
        in_maps.append({"blob": blob})
    return in_maps




class _Runner:
    """Persistent PJRT runner: traces/compiles once, keeps inputs on device,
    supports steady-state timing of repeated executions."""

    def __init__(self, nc, in_maps):
        import jax
        from jax.sharding import Mesh, PartitionSpec
        try:
            from jax.experimental.shard_map import shard_map
        except ImportError:
            from jax.shard_map import shard_map
        from concourse import bass2jax, mybir as mb

        bass2jax.install_neuronx_cc_hook()
        self.jax = jax
        partition_name = (
            nc.partition_id_tensor.name if nc.partition_id_tensor else None
        )
        in_names, out_names, out_avals, zero_outs = [], [], [], []
        for alloc in nc.m.functions[0].allocations:
            if not isinstance(alloc, mb.MemoryLocationSet):
                continue
            name = alloc.memorylocations[0].name
            if alloc.kind == "ExternalInput":
                if name != partition_name:
                    in_names.append(name)
            elif alloc.kind == "ExternalOutput":
                out_names.append(name)
                shape = tuple(alloc.tensor_shape)
                dtype = mb.dt.np(alloc.dtype)
                out_avals.append(jax.core.ShapedArray(shape, dtype))
                zero_outs.append(np.zeros(shape, dtype))
        n_params = len(in_names)
        all_names = in_names + out_names
        if partition_name is not None:
            all_names.append(partition_name)

        def _body(*args):
            operands = list(args)
            if partition_name is not None:
                operands.append(bass2jax.partition_id_tensor())
            outs = bass2jax._bass_exec_p.bind(
                *operands,
                out_avals=tuple(out_avals),
                in_names=tuple(all_names),
                out_names=tuple(out_names),
                lowering_input_output_aliases=(),
                sim_require_finite=True,
                sim_require_nnan=True,
                nc=nc,
            )
            return tuple(outs)

        devices = jax.devices()[:NCORES]
        mesh = Mesh(np.asarray(devices), ("core",))
        n_outs = len(out_avals)
        self.fn = jax.jit(
            shard_map(
                _body,
                mesh=mesh,
                in_specs=(PartitionSpec("core"),) * (n_params + n_outs),
                out_specs=(PartitionSpec("core"),) * n_outs,
                check_rep=False,
            ),
            keep_unused=True,
        )
        self.out_names = out_names
        self.out_avals = out_avals
        concat_in = [
            np.concatenate([np.asarray(in_maps[c][nm]) for c in range(NCORES)])
            for nm in in_names
        ]
        concat_zeros = [
            np.concatenate([z] * NCORES, axis=0) for z in zero_outs
        ]
        self.dev_args = [jax.device_put(a) for a in concat_in + concat_zeros]
        self.update_idx = {nm: i for i, nm in enumerate(in_names)}
        self.in_names = in_names

    def refresh(self, in_maps):
        for nm in self.in_names:
            arr = np.concatenate(
                [np.asarray(in_maps[c][nm]) for c in range(NCORES)]
            )
            self.dev_args[self.update_idx[nm]] = self.jax.device_put(arr)

    def update_input(self, name, per_core_arrays):
        arr = np.concatenate([np.asarray(a) for a in per_core_arrays])
        self.dev_args[self.update_idx[name]] = self.jax.device_put(arr)

    def run(self):
        outs = self.fn(*self.dev_args)
        self.jax.block_until_ready(outs)
        return [
            {
                nm: np.asarray(outs[i]).reshape(NCORES, *self.out_avals[i].shape)[c]
                for i, nm in enumerate(self.out_names)
            }
            for c in range(NCORES)
        ]

    def time(self, reps=20, warmup=2):
        import time as _time
        for _ in range(warmup):
            self.jax.block_until_ready(self.fn(*self.dev_args))
        t0 = _time.time()
        outs = None
        for _ in range(reps):
            outs = self.fn(*self.dev_args)
        self.jax.block_until_ready(outs)
        return (_time.time() - t0) / reps


def kernel(x, edge_index, Wl1, bl1, Wr1, g1, b1, Wl2, bl2, Wr2, g2, b2,
           Wl3, bl3, Wr3, gc, bc, Wc, bcls):
    x = np.asarray(x)
    edge_index = np.asarray(edge_index)
    runner = get_runner(x, edge_index, Wl1, bl1, Wr1, g1, b1, Wl2, bl2, Wr2,
                        g2, b2, Wl3, bl3, Wr3, gc, bc, Wc, bcls)
    results = runner.run()
    out = np.empty((N, NCLS), np.float32)
    for c in range(NCORES):
        out[c * NPC : (c + 1) * NPC] = results[c]["out"][:, :NPC].T.astype(np.float32)
    return out


def get_runner(x, edge_index, Wl1, bl1, Wr1, g1, b1, Wl2, bl2, Wr2, g2, b2,
               Wl3, bl3, Wr3, gc, bc, Wc, bcls):
    x = np.asarray(x)
    edge_index = np.asarray(edge_index)
    sched = _schedule(edge_index)
    key = (sched["TL"], sched["TH"], tuple(sched["ntiles"].ravel().tolist()))
    if key not in _cache:
        _cache[key] = _build(sched)
    nc = _cache[key]
    weights = (Wl1, bl1, Wr1, g1, b1, Wl2, bl2, Wr2, g2, b2,
               Wl3, bl3, Wr3, gc, bc, Wc, bcls)
    in_maps = _prep_inputs(x, sched, [np.asarray(w) for w in weights])
    rkey = ("runner", key)
    if rkey not in _cache:
        _cache[rkey] = _Runner(nc, in_maps)
    else:
        _cache[rkey].refresh(in_maps)
    return _cache[rkey]


# revision 20
# speedup vs baseline: 1.0696x; 1.0696x over previous
"""Trainium2 Bass kernel for a 3-layer GraphSAGE GNN (CellTypeGNN).

Strategy (graph/data parallel over 8 NeuronCores):
- Nodes are sharded by range: core c owns nodes [c*6250, (c+1)*6250).
- Edges are assigned to the core owning their destination node, grouped into
  128-node destination windows, and packed into 128-edge subtiles.
- Messages x[src] are fetched with batched SWDGE dma_gather (fp16, 256B rows)
  from a full replica of the current features in DRAM, built on device by an
  AllGather of the per-core fp16 node shard (layer 1 included, so the host
  only ever uploads the local shard). int16 gather indices only reach 32767,
  so the table is split into lo (nodes < 25000) and hi halves; edges are
  segregated by source half within each window.
- Segment-mean aggregation: for each 128-edge subtile an fp16 one-hot matrix
  scaled by 1/deg(dst) is built on the vector engine with a single
  tensor_scalar(is_equal, mult); PE matmuls msg^T @ onehot accumulate the mean
  into PSUM per destination window, feature-major.
- SAGE linear: fp16 weight matmuls (Wl on aggregated mean + Wr on x) into the
  same PSUM bank; LayerNorm stats computed node-major (PE transpose),
  affine+GELU fused into one scalar-engine activation, residual on DVE.
- After layers 1 and 2 the updated fp16 node features are AllGathered across
  the 8 cores so the next layer can gather arbitrary source nodes.
- Classifier LayerNorm affine is folded into the final linear on the host.
- Per-call host->device staging cost in this environment is dominated by the
  NUMBER of runtime input buffers (~1.7 ms each), so ALL per-core inputs are
  packed into one int16 blob (x shard fp16, compact 16-row wrapped gather
  indices, rel/inv edge tables fp16, packed weights) and unpacked on device:
  indices are replicated 16->128 rows with DRAM->DRAM DMAs, rel/inv converted
  to fp32 on DVE, iota/identity matrices generated with gpsimd iota.
"""

import numpy as np
from contextlib import ExitStack

import concourse.bass as bass
import concourse.tile as tile
from concourse import bacc, mybir
from concourse.bass_utils import run_bass_kernel_spmd

P = 128
N, E, D = 50000, 800000, 128
DOUT, NCLS = 64, 40
NCORES = 8
NPC = N // NCORES            # 6250 nodes per core
W = (NPC + P - 1) // P       # 49 windows per core
NPAD = W * P                 # 6272 padded per-core node count
HALF = 25000                 # lo/hi table split (int16 gather indices)
G = 32                       # gather chunk size in 128-edge subtiles
LN_EPS = 1e-5
W16COLS = 4 * 128 + 2 * 64 + NCLS   # packed fp16 weights: wl0 wr0 wl1 wr1 wl2 wr2 wc
W32COLS = 9                          # bl0 bl1 bl2 g0 b0 g1 b1 bc eps

f32 = mybir.dt.float32
f16 = mybir.dt.float16
i16 = mybir.dt.int16
i8 = mybir.dt.int8

_cache = {}


def _layout(TL, TH):
    """Blob segment layout in int16 elements. Every segment size is a
    multiple of 64 so offsets stay 128B aligned."""
    T2 = TL + TH
    segs = {}
    off = 0

    def add(name, rows, cols, dt):
        nonlocal off
        if dt == f32:
            n = rows * cols * 2
        elif dt == i8:
            n = (rows * cols + 1) // 2
        else:
            n = rows * cols
        segs[name] = (off, rows, cols, dt)
        off += (n + 63) & ~63

    add("xsh", NPC, D, f16)
    add("idxA", 16, TL * 8, i16)
    add("idxB", 16, TH * 8, i16)
    add("rel8", P, T2, i8)
    add("inv16", P, T2, f16)
    add("wts16", P, W16COLS, f16)
    add("wts32", P, W32COLS, f32)
    segs["total"] = off
    return segs


def _schedule(edge_index):
    """Host-side edge preprocessing. Returns per-core gather/one-hot arrays
    plus the SPMD-uniform window subtile schedule."""
    src = edge_index[0].astype(np.int64)
    dst = edge_index[1].astype(np.int64)
    deg = np.bincount(dst, minlength=N)
    invdeg_all = (1.0 / np.maximum(deg, 1)).astype(np.float32)

    core = dst // NPC
    loc = dst - core * NPC
    win = loc >> 7
    rel = (loc & 127).astype(np.float32)
    half = (src >= HALF).astype(np.int64)

    # group id: (core, window, half); edges sorted by group
    gid = (core * W + win) * 2 + half
    order = np.argsort(gid, kind="stable")
    gid_s = gid[order]
    counts = np.bincount(gid_s, minlength=NCORES * W * 2).reshape(NCORES, W, 2)

    # SPMD-uniform subtile counts per (window, half)
    ntiles = (counts.max(axis=0) + P - 1) // P  # [W, 2]
    empty = ntiles.sum(axis=1) == 0
    ntiles[empty, 0] = 1
    TL = int(ntiles[:, 0].sum())
    TH = int(ntiles[:, 1].sum())
    startA = np.concatenate([[0], np.cumsum(ntiles[:, 0])[:-1]]).astype(np.int64)
    startB = np.concatenate([[0], np.cumsum(ntiles[:, 1])[:-1]]).astype(np.int64)

    # per-core stream arrays
    idxA = np.zeros((NCORES, P, TL), np.int16)
    idxB = np.zeros((NCORES, P, TH), np.int16)
    relA = np.full((NCORES, P, TL), -1.0, np.float32)
    relB = np.full((NCORES, P, TH), -1.0, np.float32)
    invA = np.zeros((NCORES, P, TL), np.float32)
    invB = np.zeros((NCORES, P, TH), np.float32)

    # vectorized placement: rank of each edge within its (core, win, half) group
    grp_start_per_edge = np.concatenate([[0], np.cumsum(np.bincount(
        gid_s, minlength=NCORES * W * 2))])[gid_s]
    rank = np.arange(len(gid_s)) - grp_start_per_edge
    e_core = core[order]
    e_win = win[order]
    e_half = half[order]
    e_src = src[order]
    e_rel = rel[order]
    e_inv = invdeg_all[dst[order]]
    pos = np.where(e_half == 0, startA[e_win], startB[e_win]) + (rank >> 7)
    prt = rank & 127

    mA = e_half == 0
    idxA[e_core[mA], prt[mA], pos[mA]] = e_src[mA].astype(np.int16)
    relA[e_core[mA], prt[mA], pos[mA]] = e_rel[mA]
    invA[e_core[mA], prt[mA], pos[mA]] = e_inv[mA]
    mB = ~mA
    idxB[e_core[mB], prt[mB], pos[mB]] = (e_src[mB] - HALF).astype(np.int16)
    relB[e_core[mB], prt[mB], pos[mB]] = e_rel[mB]
    invB[e_core[mB], prt[mB], pos[mB]] = e_inv[mB]

    def wrap16(idx_pt):  # [P, T] slot-major -> compact wrapped [16, T*8]
        Tn = idx_pt.shape[1]
        out = np.zeros((16, Tn * 8), np.int16)
        for c0 in range(0, Tn, G):
            c1 = min(c0 + G, Tn)
            flat = idx_pt[:, c0:c1].T.reshape(-1)  # i = t_local*128 + p
            w16 = flat.reshape(-1, 16).T  # [16, n/16]: i -> [i%16, i//16]
            out[:, c0 * 8 : c0 * 8 + w16.shape[1]] = w16
        return out

    idxAw = np.stack([wrap16(idxA[c]) for c in range(NCORES)])
    idxBw = np.stack([wrap16(idxB[c]) for c in range(NCORES)])
    return dict(
        ntiles=ntiles, TL=TL, TH=TH, startA=startA, startB=startB,
        idxA=idxAw, idxB=idxBw, relA=relA, relB=relB, invA=invA, invB=invB,
    )


def _build(sched, krep=1):
    """Build and compile the SPMD Bass program."""
    ntiles, TL, TH = sched["ntiles"], sched["TL"], sched["TH"]
    startA, startB = sched["startA"], sched["startB"]
    T2 = TL + TH
    lay = _layout(TL, TH)

    nc = bacc.Bacc("TRN2", target_bir_lowering=False, debug=False,
                   num_devices=NCORES)

    blob_d = nc.dram_tensor("blob", [lay["total"]], i16, kind="ExternalInput")
    out_d = nc.dram_tensor("out", [NCLS, NPAD], f16, kind="ExternalOutput")

    idxA_rep = nc.dram_tensor("idxA_rep", [P, TL * 8], i16)
    idxB_rep = nc.dram_tensor("idxB_rep", [P, TH * 8], i16)
    xg_sh = nc.dram_tensor("xg_sh", [NPC, D], f16)
    xg_fullx = nc.dram_tensor("xg_fullx", [N, D], f16, addr_space="Shared")
    xg_own = [nc.dram_tensor(f"xg{l}_own", [NPC, D], f16) for l in range(2)]
    xg_full = [
        nc.dram_tensor(f"xg{l}_full", [N, D], f16, addr_space="Shared")
        for l in range(2)
    ]

    def seg_ap(name):
        off, rows, cols, dt = lay[name]
        if dt == f32:
            n = rows * cols * 2
        elif dt == i8:
            n = (rows * cols + 1) // 2
        else:
            n = rows * cols
        a = blob_d.ap()[off : off + n]
        if dt != i16:
            a = a.bitcast(dt)
        return a.rearrange("(r c) -> r c", c=cols)

    with tile.TileContext(nc) as tc, ExitStack() as ctx:
        cpool = ctx.enter_context(tc.tile_pool(name="const", bufs=1))
        xpool = ctx.enter_context(tc.tile_pool(name="x", bufs=1))
        idxpool = ctx.enter_context(tc.tile_pool(name="idx", bufs=4))
        msgApool = ctx.enter_context(tc.tile_pool(name="msgA", bufs=4))
        msgBpool = ctx.enter_context(tc.tile_pool(name="msgB", bufs=4))
        ohpool = ctx.enter_context(tc.tile_pool(name="oh", bufs=16))
        wkpool = ctx.enter_context(tc.tile_pool(name="wk", bufs=4))
        stpool = ctx.enter_context(tc.tile_pool(name="st", bufs=8))
        psA = ctx.enter_context(tc.tile_pool(name="psA", bufs=3, space="PSUM"))
        psC = ctx.enter_context(tc.tile_pool(name="psC", bufs=2, space="PSUM"))
        psT = ctx.enter_context(tc.tile_pool(name="psT", bufs=3, space="PSUM"))

        # ---- preamble: unpack the blob --------------------------------
        # layer-1 gather table: AllGather the fp16 shard
        nc.sync.dma_start(out=xg_sh.ap(), in_=seg_ap("xsh"))
        nc.gpsimd.collective_compute(
            "AllGather",
            mybir.AluOpType.bypass,
            replica_groups=[list(range(NCORES))],
            ins=[xg_sh.ap()],
            outs=[xg_fullx.ap()],
        )

        # replicate compact 16-row wrapped indices to the 128-row form the
        # SWDGE gather expects (DRAM->DRAM)
        segA, segB = seg_ap("idxA"), seg_ap("idxB")
        for k in range(8):
            nc.sync.dma_start(out=idxA_rep.ap()[16 * k : 16 * (k + 1), :], in_=segA)
            nc.sync.dma_start(out=idxB_rep.ap()[16 * k : 16 * (k + 1), :], in_=segB)

        # packed weights
        w16 = cpool.tile([P, W16COLS], f16, name="w16")
        nc.sync.dma_start(out=w16[:], in_=seg_ap("wts16"))
        w32 = cpool.tile([P, W32COLS], f32, name="w32")
        nc.sync.dma_start(out=w32[:], in_=seg_ap("wts32"))
        wl_v = [w16[:, 0:128], w16[:, 256:384], w16[:, 512:576]]
        wr_v = [w16[:, 128:256], w16[:, 384:512], w16[:, 576:640]]
        wc_v = w16[:DOUT, 640 : 640 + NCLS]
        bl_v = [w32[:, 0:1], w32[:, 1:2], w32[:DOUT, 2:3]]
        g_v = [w32[:, 3:4], w32[:, 5:6]]
        b_v = [w32[:, 4:5], w32[:, 6:7]]
        bc_v = w32[:NCLS, 7:8]
        eps_v = w32[:, 8:9]

        # rel/inv tables: int8/fp16 upload -> fp32 working copy
        r8 = cpool.tile([P, T2], i8, name="r8")
        nc.sync.dma_start(out=r8[:], in_=seg_ap("rel8"))
        iv16 = cpool.tile([P, T2], f16, name="iv16")
        nc.sync.dma_start(out=iv16[:], in_=seg_ap("inv16"))
        conv = cpool.tile([P, 2 * T2], f32, name="conv")
        nc.any.tensor_copy(conv[:, :T2], r8[:])
        nc.any.tensor_copy(conv[:, T2:], iv16[:])
        # conv columns: [relA TL][relB TH][invA TL][invB TH]

        # iota / identity constants, generated on device
        iota16 = cpool.tile([P, P], f16, name="iota16")
        nc.gpsimd.iota(iota16[:], [[1, P]], channel_multiplier=0,
                       allow_small_or_imprecise_dtypes=True)
        iota32 = cpool.tile([P, P], f32, name="iota32")
        nc.gpsimd.iota(iota32[:], [[1, P]], channel_multiplier=0,
                       allow_small_or_imprecise_dtypes=True)
        pidx = cpool.tile([P, 1], f32, name="pidx")
        nc.gpsimd.iota(pidx[:], [[1, 1]], channel_multiplier=1,
                       allow_small_or_imprecise_dtypes=True)
        ident32 = cpool.tile([P, P], f32, name="ident32")
        nc.vector.tensor_scalar(out=ident32[:], in0=iota32[:],
                                scalar1=pidx[:, :1], scalar2=None,
                                op0=mybir.AluOpType.is_equal)

        # feature-major working copies of the local shard
        xfm32 = xpool.tile([P, NPAD], f32, tag="xfm32", name="xfm32")
        xfm16 = xpool.tile([P, NPAD], f16, tag="xfm16", name="xfm16")
        xsha = seg_ap("xsh")
        for w in range(W):
            rows = min(P, NPC - w * P)
            nm = wkpool.tile([P, P], f16, tag="xnm_in")
            nc.sync.dma_start(out=nm[:rows, :], in_=xsha[w * P : w * P + rows, :])
            nm32 = wkpool.tile([P, P], f32, tag="xnm32")
            nc.any.tensor_copy(nm32[:rows, :], nm[:rows, :])
            tp = psT.tile([P, P], f32, space="PSUM", tag="tp")
            nc.tensor.transpose(tp[:, :rows], nm32[:rows, :], ident32[:rows, :rows])
            nc.any.tensor_copy(xfm32[:, w * P : w * P + rows], tp[:, :rows])
            nc.any.tensor_copy(xfm16[:, w * P : w * P + rows], tp[:, :rows])
        if NPC < NPAD:
            nc.vector.memset(xfm32[:, NPC:NPAD], 0)
            nc.vector.memset(xfm16[:, NPC:NPAD], 0)

        x3fm = xpool.tile([DOUT, NPAD], f32, tag="x3fm")
        normfm = xpool.tile([DOUT, NPAD], f16, tag="normfm")

        nchunkA = (TL + G - 1) // G
        nchunkB = (TH + G - 1) // G

        for l in list(range(3)) * krep:
            dout = D if l < 2 else DOUT
            cur32, cur16 = xfm32, xfm16
            nxt32, nxt16 = xfm32, xfm16
            gsrc = xg_fullx if l == 0 else xg_full[l - 1]
            src_lo = gsrc.ap()[:HALF, :]
            src_hi = gsrc.ap()[HALF:, :]

            # emit gather chunks lazily; Tile pool backpressure pipelines them
            msgs = {"A": {}, "B": {}}
            issued = {"A": -1, "B": -1}

            def emit_chunk(stream, ci, l=l, src_lo=src_lo, src_hi=src_hi,
                           msgs=msgs):
                Tn = TL if stream == "A" else TH
                idxd = idxA_rep if stream == "A" else idxB_rep
                mpool = msgApool if stream == "A" else msgBpool
                src = src_lo if stream == "A" else src_hi
                c0 = ci * G
                cn = min(G, Tn - c0)
                nidx = cn * P
                it = idxpool.tile([P, G * 8], i16, tag="idx")
                nc.sync.dma_start(
                    out=it[:, : cn * 8], in_=idxd.ap()[:, c0 * 8 : c0 * 8 + cn * 8]
                )
                mt = mpool.tile([P, G * P], f16, tag=f"msg{stream}")
                nc.gpsimd.dma_gather(
                    mt[:, : cn * P].rearrange("p (t d) -> p t d", d=P),
                    src,
                    it[:, : cn * 8],
                    nidx,
                    nidx,
                    P,
                    single_packet=False,
                )
                msgs[stream][ci] = mt

            for w in range(W):
                nA, nB = int(ntiles[w, 0]), int(ntiles[w, 1])
                subs = [("A", int(startA[w]) + i) for i in range(nA)] + [
                    ("B", int(startB[w]) + i) for i in range(nB)
                ]
                for stream, pos in subs:
                    while issued[stream] < pos // G:
                        issued[stream] += 1
                        emit_chunk(stream, issued[stream])

                ps = psA.tile([P, P], f32, space="PSUM", tag="agg")
                for si, (stream, pos) in enumerate(subs):
                    rcol = pos if stream == "A" else TL + pos
                    icol = T2 + rcol
                    mt = msgs[stream][pos // G]
                    t = pos % G
                    oh = ohpool.tile([P, P], f16, tag="oh")
                    nc.vector.tensor_scalar(
                        out=oh[:],
                        in0=iota16[:],
                        scalar1=conv[:, rcol : rcol + 1],
                        scalar2=conv[:, icol : icol + 1],
                        op0=mybir.AluOpType.is_equal,
                        op1=mybir.AluOpType.mult,
                    )
                    nc.tensor.matmul(
                        out=ps[:],
                        lhsT=mt[:, t * P : (t + 1) * P],
                        rhs=oh[:],
                        start=(si == 0),
                        stop=(si == len(subs) - 1),
                    )
                agg16 = wkpool.tile([P, P], f16, tag="agg16")
                nc.any.tensor_copy(agg16[:], ps[:])

                hps = psC.tile([dout, P], f32, space="PSUM", tag="h")
                nc.tensor.matmul(out=hps[:], lhsT=wl_v[l], rhs=agg16[:],
                                 start=True, stop=False)
                nc.tensor.matmul(out=hps[:], lhsT=wr_v[l],
                                 rhs=cur16[:, w * P : (w + 1) * P],
                                 start=False, stop=True)

                cols = slice(w * P, (w + 1) * P)
                if l < 2:
                    hfm = wkpool.tile([P, P], f32, tag="hfm")
                    nc.scalar.activation(hfm[:], hps[:],
                                         mybir.ActivationFunctionType.Identity,
                                         bias=bl_v[l])
                    tp1 = psT.tile([P, P], f32, space="PSUM", tag="tp")
                    nc.tensor.transpose(tp1[:], hfm[:], ident32[:])
                    s_ = stpool.tile([P, 1], f32, tag="sum")
                    nc.vector.reduce_sum(s_[:], tp1[:], axis=mybir.AxisListType.X)
                    nmu = stpool.tile([P, 1], f32, tag="nmu")
                    nc.scalar.mul(nmu[:], s_[:], -1.0 / D)
                    xc = wkpool.tile([P, P], f32, tag="xc")
                    nc.scalar.activation(xc[:], tp1[:],
                                         mybir.ActivationFunctionType.Identity,
                                         bias=nmu[:, :1])
                    sq = wkpool.tile([P, P], f32, tag="sq")
                    ss = stpool.tile([P, 1], f32, tag="ss")
                    nc.scalar.activation(sq[:], xc[:],
                                         mybir.ActivationFunctionType.Square,
                                         accum_out=ss[:, :1])
                    sd = stpool.tile([P, 1], f32, tag="sd")
                    nc.scalar.activation(sd[:], ss[:],
                                         mybir.ActivationFunctionType.Sqrt,
                                         scale=1.0 / D, bias=eps_v)
                    rs = stpool.tile([P, 1], f32, tag="rs")
                    nc.vector.reciprocal(rs[:], sd[:])
                    nrm = wkpool.tile([P, P], f32, tag="nrm")
                    nc.vector.tensor_scalar_mul(nrm[:], xc[:], rs[:, :1])
                    tp2 = psT.tile([P, P], f32, space="PSUM", tag="tp")
                    nc.tensor.transpose(tp2[:], nrm[:], ident32[:])
                    gel = wkpool.tile([P, P], f32, tag="gel")
                    nc.scalar.activation(gel[:], tp2[:],
                                         mybir.ActivationFunctionType.Gelu,
                                         bias=b_v[l], scale=g_v[l])
                    nc.vector.tensor_add(nxt32[:, cols], gel[:], cur32[:, cols])
                    nc.any.tensor_copy(nxt16[:, cols], nxt32[:, cols])
                    tp3 = psT.tile([P, P], f32, space="PSUM", tag="tp")
                    nc.tensor.transpose(tp3[:], nxt32[:, cols], ident32[:])
                    xnm = wkpool.tile([P, P], f16, tag="xnm")
                    nc.any.tensor_copy(xnm[:], tp3[:])
                    rows = min(P, NPC - w * P)
                    nc.sync.dma_start(
                        out=xg_own[l].ap()[w * P : w * P + rows, :],
                        in_=xnm[:rows, :],
                    )
                else:
                    nc.scalar.activation(x3fm[:, cols], hps[:],
                                         mybir.ActivationFunctionType.Gelu,
                                         bias=bl_v[l])

            if l < 2:
                nc.gpsimd.collective_compute(
                    "AllGather",
                    mybir.AluOpType.bypass,
                    replica_groups=[list(range(NCORES))],
                    ins=[xg_own[l].ap()],
                    outs=[xg_full[l].ap()],
                )

        # classifier: LN (affine folded into wc) then linear
        for w in range(W):
            cols = slice(w * P, (w + 1) * P)
            tp1 = psT.tile([P, DOUT], f32, space="PSUM", tag="tp")
            nc.tensor.transpose(tp1[:], x3fm[:, cols], ident32[:DOUT, :DOUT])
            s_ = stpool.tile([P, 1], f32, tag="sum")
            nc.vector.reduce_sum(s_[:], tp1[:], axis=mybir.AxisListType.X)
            nmu = stpool.tile([P, 1], f32, tag="nmu")
            nc.scalar.mul(nmu[:], s_[:], -1.0 / DOUT)
            xc = wkpool.tile([P, DOUT], f32, tag="xc")
            nc.scalar.activation(xc[:], tp1[:],
                                 mybir.ActivationFunctionType.Identity,
                                 bias=nmu[:, :1])
            sq = wkpool.tile([P, DOUT], f32, tag="sq")
            ss = stpool.tile([P, 1], f32, tag="ss")
            nc.scalar.activation(sq[:], xc[:],
                                 mybir.ActivationFunctionType.Square,
                                 accum_out=ss[:, :1])
            sd = stpool.tile([P, 1], f32, tag="sd")
            nc.scalar.activation(sd[:], ss[:],
                                 mybir.ActivationFunctionType.Sqrt,
                                 scale=1.0 / DOUT, bias=eps_v)
            rs = stpool.tile([P, 1], f32, tag="rs")
            nc.vector.reciprocal(rs[:], sd[:])
            nrm = wkpool.tile([P, DOUT], f32, tag="nrm")
            nc.vector.tensor_scalar_mul(nrm[:], xc[:], rs[:, :1])
            tp2 = psT.tile([DOUT, P], f32, space="PSUM", tag="tp")
            nc.tensor.transpose(tp2[:], nrm[:], ident32[:])
            nc.any.tensor_copy(normfm[:, cols], tp2[:])

        NCHUNK = 512
        for c0 in range(0, NPAD, NCHUNK):
            cn = min(NCHUNK, NPAD - c0)
            ops = psC.tile([NCLS, NCHUNK], f32, space="PSUM", tag="h")
            nc.tensor.matmul(out=ops[:, :cn], lhsT=wc_v,
                             rhs=normfm[:, c0 : c0 + cn], start=True, stop=True)
            osb = wkpool.tile([NCLS, NCHUNK], f16, tag="osb")
            nc.scalar.activation(osb[:, :cn], ops[:, :cn],
                                 mybir.ActivationFunctionType.Identity,
                                 bias=bc_v)
            nc.sync.dma_start(out=out_d.ap()[:, c0 : c0 + cn], in_=osb[:, :cn])

    nc.compile()
    return nc


def _prep_inputs(x, sched, weights):
    """Pack per-core input blobs."""
    TL, TH = sched["TL"], sched["TH"]
    T2 = TL + TH
    lay = _layout(TL, TH)
    (Wl1, bl1, Wr1, g1, b1, Wl2, bl2, Wr2, g2, b2,
     Wl3, bl3, Wr3, gc, bc, Wc, bcls) = weights
    wcp = (gc[:, None].astype(np.float32) * Wc.astype(np.float32))
    bcp = bc.astype(np.float32) @ Wc.astype(np.float32) + bcls.astype(np.float32)

    wts16 = np.zeros((P, W16COLS), np.float16)
    wts16[:, 0:128] = Wl1.astype(np.float16)
    wts16[:, 128:256] = Wr1.astype(np.float16)
    wts16[:, 256:384] = Wl2.astype(np.float16)
    wts16[:, 384:512] = Wr2.astype(np.float16)
    wts16[:, 512:576] = Wl3.astype(np.float16)
    wts16[:, 576:640] = Wr3.astype(np.float16)
    wts16[:DOUT, 640 : 640 + NCLS] = wcp.astype(np.float16)

    wts32 = np.zeros((P, W32COLS), np.float32)
    wts32[:, 0] = bl1.astype(np.float32)
    wts32[:, 1] = bl2.astype(np.float32)
    wts32[:DOUT, 2] = bl3.astype(np.float32)
    wts32[:, 3] = g1.astype(np.float32)
    wts32[:, 4] = b1.astype(np.float32)
    wts32[:, 5] = g2.astype(np.float32)
    wts32[:, 6] = b2.astype(np.float32)
    wts32[:NCLS, 7] = bcp
    wts32[:, 8] = LN_EPS

    def place(blob, name, arr):
        off = lay[name][0]
        a = arr.reshape(-1).view(np.int16)
        blob[off : off + a.size] = a

    in_maps = []
    for c in range(NCORES):
        blob = np.zeros(lay["total"], np.int16)
        place(blob, "xsh", np.ascontiguousarray(
            x[c * NPC : (c + 1) * NPC].astype(np.float16)))
        place(blob, "idxA", sched["idxA"][c])
        place(blob, "idxB", sched["idxB"][c])
        rel8 = np.empty((P, T2), np.int8)
        rel8[:, 0:TL] = sched["relA"][c]
        rel8[:, TL:T2] = sched["relB"][c]
        place(blob, "rel8", rel8)
        inv16 = np.empty((P, T2), np.float16)
        inv16[:, 0:TL] = sched["invA"][c]
        inv16[:, TL:T2] = sched["invB"][c]
        place(blob, "inv16", inv16)
        place(blob, "wts16", wts16)
        place(blob, "wts32", wts32)
        in_maps.append({"blob": blob})
    return in_maps




class _Runner:
    """Persistent PJRT runner: traces/compiles once, keeps inputs on device,
    supports steady-state timing of repeated executions."""

    def __init__(self, nc, in_maps):
        import jax
        from jax.sharding import Mesh, PartitionSpec
        try:
            from jax.experimental.shard_map import shard_map
        except ImportError:
            from jax.shard_map import shard_map
        from concourse import bass2jax, mybir as mb

        bass2jax.install_neuronx_cc_hook()
        self.jax = jax
        partition_name = (
            nc.partition_id_tensor.name if nc.partition_id_tensor else None
        )
        in_names, out_names, out_avals, zero_outs = [], [], [], []
        for alloc in nc.m.functions[0].allocations:
            if not isinstance(alloc, mb.MemoryLocationSet):
                continue
            name = alloc.memorylocations[0].name
            if alloc.kind == "ExternalInput":
                if name != partition_name:
                    in_names.append(name)
            elif alloc.kind == "ExternalOutput":
                out_names.append(name)
                shape = tuple(alloc.tensor_shape)
                dtype = mb.dt.np(alloc.dtype)
                out_avals.append(jax.core.ShapedArray(shape, dtype))
                zero_outs.append(np.zeros(shape, dtype))
        n_params = len(in_names)
        all_names = in_names + out_names
        if partition_name is not None:
            all_names.append(partition_name)

        def _body(*args):
            operands = list(args)
            if partition_name is not None:
                operands.append(bass2jax.partition_id_tensor())
            outs = bass2jax._bass_exec_p.bind(
                *operands,
                out_avals=tuple(out_avals),
                in_names=tuple(all_names),
                out_names=tuple(out_names),
                lowering_input_output_aliases=(),
                sim_require_finite=True,
                sim_require_nnan=True,
                nc=nc,
            )
            return tuple(outs)

        devices = jax.devices()[:NCORES]
        mesh = Mesh(np.asarray(devices), ("core",))
        n_outs = len(out_avals)
        self.fn = jax.jit(
            shard_map(
                _body,
                mesh=mesh,
                in_specs=(PartitionSpec("core"),) * (n_params + n_outs),
                out_specs=(PartitionSpec("core"),) * n_outs,
                check_rep=False,
            ),
            keep_unused=True,
        )
        self.out_names = out_names
        self.out_avals = out_avals
        concat_in = [
            np.concatenate([np.asarray(in_maps[c][nm]) for c in range(NCORES)])
            for nm in in_names
        ]
        concat_zeros = [
            np.concatenate([z] * NCORES, axis=0) for z in zero_outs
        ]
        self.dev_args = [jax.device_put(a) for a in concat_in + concat_zeros]
        self.update_idx = {nm: i for i, nm in enumerate(in_names)}
        self.in_names = in_names

    def refresh(self, in_maps):
        for nm in self.in_names:
            arr = np.concatenate(
                [np.asarray(in_maps[c][nm]) for c in range(NCORES)]
            )
            self.dev_args[self.update_idx[nm]] = self.jax.device_put(arr)

    def update_input(self, name, per_core_arrays):
        arr = np.concatenate([np.asarray(a) for a in per_core_arrays])
        self.dev_args[self.update_idx[name]] = self.jax.device_put(arr)

    def run(self):
        outs = self.fn(*self.dev_args)
        self.jax.block_until_ready(outs)
        return [
            {
                nm: np.asarray(outs[i]).reshape(NCORES, *self.out_avals[i].shape)[c]
                for i, nm in enumerate(self.out_names)
            }
            for c in range(NCORES)
        ]

    def time(self, reps=20, warmup=2):
        import time as _time
        for _ in range(warmup):
            self.jax.block_until_ready(self.fn(*self.dev_args))
        t0 = _time.time()
        outs = None
        for _ in range(reps):
            outs = self.fn(*self.dev_args)
        self.jax.block_until_ready(outs)
        return (_time.time() - t0) / reps


def kernel(x, edge_index, Wl1, bl1, Wr1, g1, b1, Wl2, bl2, Wr2, g2, b2,
           Wl3, bl3, Wr3, gc, bc, Wc, bcls):
    x = np.asarray(x)
    edge_index = np.asarray(edge_index)
    runner = get_runner(x, edge_index, Wl1, bl1, Wr1, g1, b1, Wl2, bl2, Wr2,
                        g2, b2, Wl3, bl3, Wr3, gc, bc, Wc, bcls)
    results = runner.run()
    out = np.empty((N, NCLS), np.float32)
    for c in range(NCORES):
        out[c * NPC : (c + 1) * NPC] = results[c]["out"][:, :NPC].T.astype(np.float32)
    return out


def get_runner(x, edge_index, Wl1, bl1, Wr1, g1, b1, Wl2, bl2, Wr2, g2, b2,
               Wl3, bl3, Wr3, gc, bc, Wc, bcls):
    x = np.asarray(x)
    edge_index = np.asarray(edge_index)
    sched = _schedule(edge_index)
    key = (sched["TL"], sched["TH"], tuple(sched["ntiles"].ravel().tolist()))
    if key not in _cache:
        _cache[key] = _build(sched)
    nc = _cache[key]
    weights = (Wl1, bl1, Wr1, g1, b1, Wl2, bl2, Wr2, g2, b2,
               Wl3, bl3, Wr3, gc, bc, Wc, bcls)
    in_maps = _prep_inputs(x, sched, [np.asarray(w) for w in weights])
    rkey = ("runner", key)
    if rkey not in _cache:
        _cache[rkey] = _Runner(nc, in_maps)
    else:
        _cache[rkey].refresh(in_maps)
    return _cache[rkey]
